# revision 2
# baseline (speedup 1.0000x reference)
"""Trainium2 Bass kernel for nn_SubspaceLinopFactory (subspace NUDFT forward op).

Math (reference):
  s[a,c,h,w] = x[a,h,w] * mps[c,h,w]
  E[r,k,(h,w)] = exp(-i*(trj[r,0,k]*gy[h] + trj[r,1,k]*gx[w]))   (separable!)
  y[a,r,c,k] = sum_hw E * s
  z[r,t,c,k] = sum_a phi[a,t] * y[a,r,c,k] * sqrt_dcf[r,k]
  out[t,c,k] = z[subsamp_idx[t], t, c, k]

Sharding: trajectory r -> core r (R == 8 == n_cores). Each core computes
z[t,c,k] for all t with its own r; host gathers rows where subsamp_idx[t]==r.

Device pipeline per core (separable NUDFT, all f32):
  1. trig tables: phase in "turns" m = ty*gy/2pi via ScalarE Copy(scale),
     range-reduce frac = m - round(m) (round-to-nearest int32 cast on VectorE),
     then ScalarE Sin(2pi*frac). Tables: Ey cos/sin [128,(2x h)|1024 k],
     Ex cos/sin [64 w, 1024 k] (the latter scaled by sqrt_dcf).
  2. stage 1 (TensorE): P[(a,c,h), k] = sum_w sT[w,ach] * (dcf*cos_x)[w,k]
     (and Q with sin_x), as 6 m-tiles of 128 x 2 k-chunks of 512.
  3. stage 2 (VectorE): products A=cy*P, B=sy*Q, C=cy*Q, D=sy*P.
  4. h-reduction (TensorE): selector matmuls (+-1 weights) contract the
     (ac,h) partitions to y_re[ac,k] = sum_h A-B, y_im = -(C+D), accumulated
     in PSUM across the 6 m-tiles.
  5. phi expansion (TensorE): z[(t,c), k] = phiT.T @ y  (rows = t*4+c = 128).
  6. out: z_re, z_im [128,1024] f32 -> host gathers into [T,C,K] complex64.
"""
import numpy as np

A, T, C, R, D, K, H, W = 3, 32, 4, 8, 2, 1024, 64, 64
N_CORES = 8
ACH = A * C * H          # 768
MT = ACH // 128          # 6 m-tiles
KC = 512                 # k-chunk (one PSUM bank of f32)
NKC = K // KC            # 2

_CACHE = {}


def _build_nc():
    import concourse.bacc as bacc
    import concourse.tile as tile
    import concourse.mybir as mybir

    AF = mybir.ActivationFunctionType
    OP = mybir.AluOpType
    F32 = mybir.dt.float32
    I32 = mybir.dt.int32
    TWO_PI = float(2 * np.pi)

    nc = bacc.Bacc(None, target_bir_lowering=False)

    d_tyr = nc.dram_tensor("tyr", [128, K], F32, kind="ExternalInput")
    d_txr = nc.dram_tensor("txr", [64, K], F32, kind="ExternalInput")
    d_pp = nc.dram_tensor("pp", [128, 2], F32, kind="ExternalInput")
    d_dcf = nc.dram_tensor("dcf", [64, K], F32, kind="ExternalInput")
    d_xr = nc.dram_tensor("xr", [64, ACH], F32, kind="ExternalInput")
    d_mr = nc.dram_tensor("mr", [64, ACH], F32, kind="ExternalInput")
    d_selp = nc.dram_tensor("selp", [128, 12 * MT], F32, kind="ExternalInput")
    d_selm = nc.dram_tensor("selm", [128, 12 * MT], F32, kind="ExternalInput")
    d_phit = nc.dram_tensor("phit", [12, 128], F32, kind="ExternalInput")
    d_zre = nc.dram_tensor("zre", [128, K], F32, kind="ExternalOutput")
    d_zim = nc.dram_tensor("zim", [128, K], F32, kind="ExternalOutput")

    with tile.TileContext(nc) as tc:
        with (
            tc.tile_pool(name="cst", bufs=1) as cst,
            tc.tile_pool(name="tabw", bufs=2) as tabw,
            tc.tile_pool(name="work", bufs=2) as work,
            tc.tile_pool(name="psA", bufs=2, space="PSUM") as psA,
            tc.tile_pool(name="psY", bufs=1, space="PSUM") as psY,
            tc.tile_pool(name="psZ", bufs=1, space="PSUM") as psZ,
        ):
            # ---- load constants ----
            tyr = cst.tile([128, K], F32)
            txr = cst.tile([64, K], F32)
            pp = cst.tile([128, 2], F32)
            dcf = cst.tile([64, K], F32)
            xr = cst.tile([64, ACH], F32)
            mr = cst.tile([64, ACH], F32)
            selp = cst.tile([128, 12 * MT], F32)
            selm = cst.tile([128, 12 * MT], F32)
            phit = cst.tile([12, 128], F32)
            for t, d in [(tyr, d_tyr), (txr, d_txr), (pp, d_pp), (dcf, d_dcf),
                         (xr, d_xr), (mr, d_mr), (selp, d_selp), (selm, d_selm),
                         (phit, d_phit)]:
                nc.gpsimd.dma_start(t[:], d[:])

            # ---- sT = x_rep * mps_rep ----
            sT = cst.tile([64, ACH], F32)
            nc.vector.tensor_tensor(sT[:], xr[:], mr[:], OP.mult)

            # ---- trig tables ----
            def trig(src, scale_ap, P, name):
                """returns (cos, sin) tiles [P, K] of sin/cos(src*scale*2pi)."""
                outs = []
                for quarter, nm in ((0.25, "c"), (0.0, "s")):
                    m = tabw.tile([P, K], F32, tag=f"m{name}{nm}")
                    nc.scalar.activation(m[:], src[:], AF.Copy,
                                         bias=quarter, scale=scale_ap)
                    mi = tabw.tile([P, K], I32, tag=f"mi{name}{nm}")
                    nc.vector.tensor_copy(mi[:], m[:])
                    mf = tabw.tile([P, K], F32, tag=f"mf{name}{nm}")
                    nc.vector.tensor_copy(mf[:], mi[:])
                    fr = tabw.tile([P, K], F32, tag=f"fr{name}{nm}")
                    nc.vector.tensor_tensor(fr[:], m[:], mf[:], OP.subtract)
                    o = cst.tile([P, K], F32, tag=f"tbl{name}{nm}")
                    nc.scalar.activation(o[:], fr[:], AF.Sin, scale=TWO_PI)
                    outs.append(o)
                return outs

            cy, sy = trig(tyr, pp[:, 0:1], 128, "y")
            cx, sx = trig(txr, pp[:64, 1:2], 64, "x")
            # fold sqrt_dcf into the x-tables
            cxd = cst.tile([64, K], F32)
            sxd = cst.tile([64, K], F32)
            nc.vector.tensor_tensor(cxd[:], cx[:], dcf[:], OP.mult)
            nc.vector.tensor_tensor(sxd[:], sx[:], dcf[:], OP.mult)

            zout_re = cst.tile([128, K], F32)
            zout_im = cst.tile([128, K], F32)

            # ---- main pipeline over k-chunks and m-tiles ----
            for kc in range(NKC):
                ks = slice(kc * KC, (kc + 1) * KC)
                yre = psY.tile([12, KC], F32, tag="yre")
                yim = psY.tile([12, KC], F32, tag="yim")
                n_acc = 2 * MT  # matmuls accumulated per yre/yim psum
                for j in range(MT):
                    js = slice(j * 128, (j + 1) * 128)
                    p_ps = psA.tile([128, KC], F32, tag="p")
                    q_ps = psA.tile([128, KC], F32, tag="q")
                    nc.tensor.matmul(p_ps[:], sT[:, js], cxd[:, ks],
                                     start=True, stop=True)
                    nc.tensor.matmul(q_ps[:], sT[:, js], sxd[:, ks],
                                     start=True, stop=True)
                    prodA = work.tile([128, KC], F32, tag="A")
                    prodB = work.tile([128, KC], F32, tag="B")
                    prodC = work.tile([128, KC], F32, tag="C")
                    prodD = work.tile([128, KC], F32, tag="D")
                    nc.vector.tensor_tensor(prodA[:], p_ps[:], cy[:, ks], OP.mult)
                    nc.vector.tensor_tensor(prodB[:], q_ps[:], sy[:, ks], OP.mult)
                    nc.vector.tensor_tensor(prodC[:], q_ps[:], cy[:, ks], OP.mult)
                    nc.vector.tensor_tensor(prodD[:], p_ps[:], sy[:, ks], OP.mult)
                    sj = slice(j * 12, (j + 1) * 12)
                    nc.tensor.matmul(yre[:], selp[:, sj], prodA[:],
                                     start=(j == 0), stop=False,
                                     skip_group_check=True)
                    nc.tensor.matmul(yre[:], selm[:, sj], prodB[:],
                                     start=False, stop=(j == MT - 1),
                                     skip_group_check=True)
                    nc.tensor.matmul(yim[:], selm[:, sj], prodC[:],
                                     start=(j == 0), stop=False,
                                     skip_group_check=True)
                    nc.tensor.matmul(yim[:], selm[:, sj], prodD[:],
                                     start=False, stop=(j == MT - 1),
                                     skip_group_check=True)
                yre_sb = work.tile([12, KC], F32, tag="yre_sb")
                yim_sb = work.tile([12, KC], F32, tag="yim_sb")
                nc.scalar.copy(yre_sb[:], yre[:])
                nc.scalar.copy(yim_sb[:], yim[:])
                zre_ps = psZ.tile([128, KC], F32, tag="zre")
                zim_ps = psZ.tile([128, KC], F32, tag="zim")
                nc.tensor.matmul(zre_ps[:], phit[:], yre_sb[:], start=True, stop=True)
                nc.tensor.matmul(zim_ps[:], phit[:], yim_sb[:], start=True, stop=True)
                nc.scalar.copy(zout_re[:, ks], zre_ps[:])
                nc.scalar.copy(zout_im[:, ks], zim_ps[:])

            nc.gpsimd.dma_start(d_zre[:], zout_re[:])
            nc.gpsimd.dma_start(d_zim[:], zout_im[:])

    nc.finalize()
    return nc


def _get_nc():
    if "nc" not in _CACHE:
        _CACHE["nc"] = _build_nc()
    return _CACHE["nc"]


def _stage_inputs(x, trj, phi, mps, sqrt_dcf):
    """Build per-core input maps (host staging: layout/replication only,
    plus tiny index/scale constants)."""
    f32 = np.float32
    gy = (np.arange(H, dtype=np.float64) - H // 2)
    inv2pi = 1.0 / (2 * np.pi)
    pp = np.zeros((128, 2), f32)
    pp[:, 0] = np.concatenate([gy, gy]) * inv2pi
    pp[:64, 1] = gy * inv2pi  # gx == gy (H == W)

    # selector matrices: block j covers ach rows [j*128,(j+1)*128);
    # partition p -> output column ac = 2*j + p//64
    selp = np.zeros((128, 12 * MT), f32)
    for j in range(MT):
        for p in range(128):
            selp[p, j * 12 + 2 * j + p // 64] = 1.0
    selm = -selp

    # phi lhsT: [a*4+c, t*4+c'] = phi[a,t] if c==c'
    phit = np.zeros((12, 128), f32)
    for a in range(A):
        for c in range(C):
            phit[a * 4 + c, c::4] = phi[a]

    # x_rep[w, (a,c,h)] = x[a,h,w];  mps_rep[w, (a,c,h)] = mps[c,h,w]
    xt = np.ascontiguousarray(x.transpose(2, 0, 1))       # [w, a, h]
    xr = np.broadcast_to(xt[:, :, None, :], (W, A, C, H)).reshape(W, ACH)
    mt = np.ascontiguousarray(mps.transpose(2, 0, 1))     # [w, c, h]
    mr = np.broadcast_to(mt[:, None, :, :], (W, A, C, H)).reshape(W, ACH)
    xr = np.ascontiguousarray(xr, dtype=f32)
    mr = np.ascontiguousarray(mr, dtype=f32)

    in_maps = []
    for r in range(N_CORES):
        ty = trj[r, 0, :].astype(f32)
        tx = trj[r, 1, :].astype(f32)
        in_maps.append({
            "tyr": np.ascontiguousarray(np.broadcast_to(ty, (128, K))),
            "txr": np.ascontiguousarray(np.broadcast_to(tx, (64, K))),
            "pp": pp,
            "dcf": np.ascontiguousarray(
                np.broadcast_to(sqrt_dcf[r].astype(f32), (64, K))),
            "xr": xr,
            "mr": mr,
            "selp": selp,
            "selm": selm,
            "phit": phit,
        })
    return in_maps


def kernel(x, trj, phi, mps, sqrt_dcf, subsamp_idx, _trace=False):
    from concourse.bass_utils import run_bass_kernel_spmd

    nc = _get_nc()
    in_maps = _stage_inputs(np.asarray(x), np.asarray(trj), np.asarray(phi),
                            np.asarray(mps), np.asarray(sqrt_dcf))
    res = run_bass_kernel_spmd(nc, in_maps, core_ids=list(range(N_CORES)),
                               trace=_trace)
    out = np.empty((T, C, K), dtype=np.complex64)
    idx = np.asarray(subsamp_idx).astype(np.int64)
    for t in range(T):
        r = int(idx[t])
        zre = res.results[r]["zre"]
        zim = res.results[r]["zim"]
        for c in range(C):
            out[t, c, :] = zre[t * 4 + c] + 1j * zim[t * 4 + c]
    if _trace:
        kernel._last_results = res
    return out


# revision 4
# speedup vs baseline: 1.6794x; 1.6794x over previous
"""Trainium2 Bass kernel for nn_SubspaceLinopFactory (subspace NUDFT forward op).

Math (reference):
  s[a,c,h,w] = x[a,h,w] * mps[c,h,w]
  E[r,k,(h,w)] = exp(-i*(trj[r,0,k]*gy[h] + trj[r,1,k]*gx[w]))   (separable)
  y[a,r,c,k] = sum_hw E * s
  z[r,t,c,k] = sum_a phi[a,t] * y[a,r,c,k] * sqrt_dcf[r,k]
  out[t,c,k] = z[subsamp_idx[t], t, c, k]

Sharding: trajectory r -> core r (R == 8 == n_cores). Each core computes
z[t,c,k] for all t with its own r; host gathers rows where subsamp_idx[t]==r.

Device pipeline per core (separable NUDFT, fp16 matmul operands / f32 accum):
  - trig tables per k-chunk: host stages packed phase inputs in "turns"
    ([sin|cos] halves; the cos half pre-shifted by a quarter turn), ScalarE
    Copy applies the per-partition gy/2pi scale, VectorE int32-cast roundtrip
    gives frac = m-round(m) in [-.5,.5], ScalarE Sin(2pi*frac) -> fp16 tables.
  - stage 1 (TensorE, fp16): P[(a,c,h),k] = sum_w sT[w,ach]*(dcf*cos_x)[w,k],
    Q likewise with sin_x. 6 m-tiles x 512-wide k-chunks, PSUM f32.
  - ScalarE casts P,Q PSUM->SBUF fp16; VectorE products A=cy*P, B=sy*Q,
    C=cy*Q, D=sy*P (fp16 2x mode).
  - h-reduction (TensorE): +-1 selector matmuls contract (ac,h) partitions:
    y_re[ac,k] = sum_h A-B, y_im = -(C+D), PSUM-accumulated over m-tiles.
  - phi expansion (TensorE): z[(t,c),k] = phiT.T @ y  (rows = t*4+c = 128).
  - z_re, z_im [128,1024] f32 -> host gathers into [T,C,K] complex64.
"""
import numpy as np

A, T, C, R, D, K, H, W = 3, 32, 4, 8, 2, 1024, 64, 64
N_CORES = 8
ACH = A * C * H          # 768
MT = ACH // 128          # 6 m-tiles
KC = 512                 # k-chunk (one PSUM bank of f32)
NKC = K // KC            # 2

_CACHE = {}


def _build_nc():
    import concourse.bacc as bacc
    import concourse.tile as tile
    import concourse.mybir as mybir

    AF = mybir.ActivationFunctionType
    OP = mybir.AluOpType
    F32 = mybir.dt.float32
    F16 = mybir.dt.float16
    I32 = mybir.dt.int32
    TWO_PI = float(2 * np.pi)

    nc = bacc.Bacc(None, target_bir_lowering=False)

    # batched inputs: big64 = [txr2 | dcf2 | xr | mr] on 64 partitions,
    # big128 = [tyr2 | pp] on 128, sel = [selp | selm] fp16, phit fp16.
    W64 = 2 * K + 2 * K + ACH + ACH  # 5632
    d_b64 = nc.dram_tensor("b64", [64, W64], F32, kind="ExternalInput")
    d_b128 = nc.dram_tensor("b128", [128, 2 * K + 2], F32, kind="ExternalInput")
    d_sel = nc.dram_tensor("sel", [128, 24 * MT], F16, kind="ExternalInput")
    d_phit = nc.dram_tensor("phit", [12, 128], F16, kind="ExternalInput")
    d_zre = nc.dram_tensor("zre", [128, K], F32, kind="ExternalOutput")
    d_zim = nc.dram_tensor("zim", [128, K], F32, kind="ExternalOutput")

    with tile.TileContext(nc) as tc:
        with (
            tc.tile_pool(name="cst", bufs=1) as cst,
            tc.tile_pool(name="tabw", bufs=2) as tabw,
            tc.tile_pool(name="tbl", bufs=2) as tblp,
            tc.tile_pool(name="work", bufs=3) as work,
            tc.tile_pool(name="psA", bufs=2, space="PSUM") as psA,
            tc.tile_pool(name="psY", bufs=1, space="PSUM") as psY,
            tc.tile_pool(name="psZ", bufs=1, space="PSUM") as psZ,
        ):
            b64 = cst.tile([64, W64], F32)
            b128 = cst.tile([128, 2 * K + 2], F32)
            sel = cst.tile([128, 24 * MT], F16)
            phit = cst.tile([12, 128], F16)
            nc.sync.dma_start(b64[:], d_b64[:])
            nc.sync.dma_start(b128[:], d_b128[:])
            nc.sync.dma_start(sel[:], d_sel[:])
            nc.sync.dma_start(phit[:], d_phit[:])

            txr2 = b64[:, 0:2 * K].rearrange("p (s k) -> p s k", s=2)
            dcf2 = b64[:, 2 * K:4 * K].rearrange("p (s k) -> p s k", s=2)
            xr = b64[:, 4 * K:4 * K + ACH]
            mr = b64[:, 4 * K + ACH:4 * K + 2 * ACH]
            tyr2 = b128[:, 0:2 * K].rearrange("p (s k) -> p s k", s=2)
            ppy = b128[:, 2 * K:2 * K + 1]
            ppx = b128[:64, 2 * K + 1:2 * K + 2]

            # sT = x_rep * mps_rep  -> fp16 [64, ACH]
            sT = cst.tile([64, ACH], F16)
            nc.vector.tensor_tensor(sT[:], xr[:], mr[:], OP.mult)

            selp = sel[:, 0:12 * MT]
            selm = sel[:, 12 * MT:24 * MT]

            zout_re = cst.tile([128, K], F32)
            zout_im = cst.tile([128, K], F32)

            def trig_chunk(src, scale_ap, P, kc, name, out_dt):
                """[P, 2, KC] fp16 table chunk: [:,0,:]=sin, [:,1,:]=cos."""
                ks = slice(kc * KC, (kc + 1) * KC)
                m = tabw.tile([P, 2, KC], F32, tag=f"m{name}")
                nc.scalar.activation(m[:], src[:, :, ks], AF.Copy, scale=scale_ap)
                mi = tabw.tile([P, 2, KC], I32, tag=f"mi{name}")
                nc.vector.tensor_copy(mi[:], m[:])
                mf = tabw.tile([P, 2, KC], F32, tag=f"mf{name}")
                nc.vector.tensor_copy(mf[:], mi[:])
                fr = tabw.tile([P, 2, KC], F32, tag=f"fr{name}")
                nc.vector.tensor_tensor(fr[:], m[:], mf[:], OP.subtract)
                o = tblp.tile([P, 2, KC], out_dt, tag=f"tbl{name}")
                nc.scalar.activation(o[:], fr[:], AF.Sin, scale=TWO_PI)
                return o

            for kc in range(NKC):
                ks = slice(kc * KC, (kc + 1) * KC)
                xt = trig_chunk(txr2, ppx, 64, kc, "x", F32)
                xtd = tblp.tile([64, 2, KC], F16, tag="xtd")
                nc.vector.tensor_tensor(xtd[:], xt[:], dcf2[:, :, ks], OP.mult)
                yt = trig_chunk(tyr2, ppy, 128, kc, "y", F16)

                yre = psY.tile([12, KC], F32, tag="yre")
                yim = psY.tile([12, KC], F32, tag="yim")
                for j in range(MT):
                    js = slice(j * 128, (j + 1) * 128)
                    p_ps = psA.tile([128, KC], F32, tag="p")
                    q_ps = psA.tile([128, KC], F32, tag="q")
                    nc.tensor.matmul(p_ps[:], sT[:, js], xtd[:, 1, :],
                                     start=True, stop=True)
                    nc.tensor.matmul(q_ps[:], sT[:, js], xtd[:, 0, :],
                                     start=True, stop=True)
                    pc = work.tile([128, KC], F16, tag="pc")
                    qc = work.tile([128, KC], F16, tag="qc")
                    nc.scalar.copy(pc[:], p_ps[:])
                    nc.scalar.copy(qc[:], q_ps[:])
                    prodA = work.tile([128, KC], F16, tag="A")
                    prodB = work.tile([128, KC], F16, tag="B")
                    prodC = work.tile([128, KC], F16, tag="C")
                    prodD = work.tile([128, KC], F16, tag="D")
                    nc.vector.tensor_tensor(prodA[:], pc[:], yt[:, 1, :], OP.mult)
                    nc.vector.tensor_tensor(prodB[:], qc[:], yt[:, 0, :], OP.mult)
                    nc.vector.tensor_tensor(prodC[:], qc[:], yt[:, 1, :], OP.mult)
                    nc.vector.tensor_tensor(prodD[:], pc[:], yt[:, 0, :], OP.mult)
                    sj = slice(j * 12, (j + 1) * 12)
                    nc.tensor.matmul(yre[:], selp[:, sj], prodA[:],
                                     start=(j == 0), stop=False,
                                     skip_group_check=True)
                    nc.tensor.matmul(yre[:], selm[:, sj], prodB[:],
                                     start=False, stop=(j == MT - 1),
                                     skip_group_check=True)
                    nc.tensor.matmul(yim[:], selm[:, sj], prodC[:],
                                     start=(j == 0), stop=False,
                                     skip_group_check=True)
                    nc.tensor.matmul(yim[:], selm[:, sj], prodD[:],
                                     start=False, stop=(j == MT - 1),
                                     skip_group_check=True)
                yre_sb = work.tile([12, KC], F16, tag="yre_sb")
                yim_sb = work.tile([12, KC], F16, tag="yim_sb")
                nc.scalar.copy(yre_sb[:], yre[:])
                nc.scalar.copy(yim_sb[:], yim[:])
                zre_ps = psZ.tile([128, KC], F32, tag="zre")
                zim_ps = psZ.tile([128, KC], F32, tag="zim")
                nc.tensor.matmul(zre_ps[:], phit[:], yre_sb[:], start=True, stop=True)
                nc.tensor.matmul(zim_ps[:], phit[:], yim_sb[:], start=True, stop=True)
                nc.scalar.copy(zout_re[:, ks], zre_ps[:])
                nc.scalar.copy(zout_im[:, ks], zim_ps[:])

            nc.gpsimd.dma_start(d_zre[:], zout_re[:])
            nc.gpsimd.dma_start(d_zim[:], zout_im[:])

    nc.finalize()
    return nc


def _get_nc():
    if "nc" not in _CACHE:
        _CACHE["nc"] = _build_nc()
    return _CACHE["nc"]


def _stage_inputs(x, trj, phi, mps, sqrt_dcf):
    """Per-core input maps. Host staging = layout/replication + tiny
    index/scale constants (phase inputs staged in 'turns' with the cos half
    pre-shifted a quarter turn; gy==0 rows use scale=1 with constant input)."""
    f32, f16 = np.float32, np.float16
    gy = np.arange(H, dtype=np.float64) - H // 2
    inv2pi = 1.0 / (2 * np.pi)

    # per-partition scales (col 0: y for 128 rows; col 1: x for 64 rows)
    sc_y = np.where(gy == 0, 1.0, gy * inv2pi)
    pp = np.zeros((128, 2), np.float64)
    pp[:, 0] = np.concatenate([sc_y, sc_y])
    pp[:64, 1] = sc_y

    # cos-half shift: ty + pi/(2*gy) so m_cos = m_sin + 1/4 turn
    with np.errstate(divide="ignore"):
        shift = np.where(gy == 0, 0.0, np.pi / (2 * gy))

    def packed_phase(tv, P):
        """[P, 2, K]: [:,0,:]=tv (sin), [:,1,:]=tv+shift (cos); gy==0 rows
        get constant 0 / 0.25 (scale is 1 there)."""
        g = np.tile(shift, P // H)
        zero = np.tile(gy == 0, P // H)
        out = np.empty((P, 2, K), np.float64)
        out[:, 0, :] = np.where(zero[:, None], 0.0, tv[None, :])
        out[:, 1, :] = np.where(zero[:, None], 0.25, tv[None, :] + g[:, None])
        return out

    # selectors: block j covers ach rows [j*128,(j+1)*128);
    # partition p -> output column ac = 2*j + p//64
    selp = np.zeros((128, 12 * MT), f16)
    for j in range(MT):
        for p in range(128):
            selp[p, j * 12 + 2 * j + p // 64] = 1.0
    sel = np.concatenate([selp, -selp], axis=1)

    phit = np.zeros((12, 128), f16)
    for a in range(A):
        for c in range(C):
            phit[a * 4 + c, c::4] = phi[a].astype(f16)

    xt = np.ascontiguousarray(x.transpose(2, 0, 1))       # [w, a, h]
    xr = np.broadcast_to(xt[:, :, None, :], (W, A, C, H)).reshape(W, ACH)
    mt = np.ascontiguousarray(mps.transpose(2, 0, 1))     # [w, c, h]
    mr = np.broadcast_to(mt[:, None, :, :], (W, A, C, H)).reshape(W, ACH)

    in_maps = []
    for r in range(N_CORES):
        ty = trj[r, 0, :].astype(np.float64)
        tx = trj[r, 1, :].astype(np.float64)
        b64 = np.empty((64, 5632), f32)
        b64[:, 0:2 * K] = packed_phase(tx, 64).reshape(64, 2 * K)
        b64[:, 2 * K:4 * K] = np.broadcast_to(
            sqrt_dcf[r].astype(f32)[None, None, :], (64, 2, K)).reshape(64, 2 * K)
        b64[:, 4 * K:4 * K + ACH] = xr
        b64[:, 4 * K + ACH:] = mr
        b128 = np.empty((128, 2 * K + 2), f32)
        b128[:, 0:2 * K] = packed_phase(ty, 128).reshape(128, 2 * K)
        b128[:, 2 * K:] = pp
        in_maps.append({"b64": b64, "b128": b128, "sel": sel, "phit": phit})
    return in_maps


def kernel(x, trj, phi, mps, sqrt_dcf, subsamp_idx, _trace=False):
    from concourse.bass_utils import run_bass_kernel_spmd

    nc = _get_nc()
    in_maps = _stage_inputs(np.asarray(x), np.asarray(trj), np.asarray(phi),
                            np.asarray(mps), np.asarray(sqrt_dcf))
    res = run_bass_kernel_spmd(nc, in_maps, core_ids=list(range(N_CORES)),
                               trace=_trace)
    out = np.empty((T, C, K), dtype=np.complex64)
    idx = np.asarray(subsamp_idx).astype(np.int64)
    for t in range(T):
        r = int(idx[t])
        zre = res.results[r]["zre"]
        zim = res.results[r]["zim"]
        for c in range(C):
            out[t, c, :] = zre[t * 4 + c] + 1j * zim[t * 4 + c]
    if _trace:
        kernel._last_results = res
    return out
